# revision 35
# baseline (speedup 1.0000x reference)
"""Trainium2 Bass kernel for the RNN-T JointNetwork problem.

  enc = h_enc @ W_enc + b_enc            (B,T,1,J)
  dec = h_dec @ W_dec                    (B,1,U,J)
  z   = tanh(enc + dec)                  (B,T,U,J)
  out = z @ W_out + b_out                (B,T,U,V)

Shapes: B=4, T=256, U=64, D=J=V=512, fp32 in/out.

Sharding: 8 cores, data parallel over (B x T/2): core c handles batch
b = c//2 and t-half th = c%2 (128 t values). Params replicated.

Final design, ~83us/core measured (vs 122us baseline); rel err 4.1e-3:
  - TRANSPOSED OUTPUT (outT[v, row]): W_out chunks stationary, zT
    moving, b_out per-partition -> evacuation via DVE tensor_scalar_add
    / ACT Identity-with-bias (5:3 split), PE does only the 256+32 main
    matmuls (TensorMatrix 99-101% busy in steady state).
  - U-MAJOR z rows within each t-group: row = u*tg + t_local.  With
    dec_repU[j, u, t] = decT[j, u] pre-replicated (one-time DVE copies
    that hide behind the setup-matmul phase), both zpre-add operands
    are innermost-step-1 bf16 APs, which unlocks the DVE 2x_1P mode
    (600ns vs 1133ns per [128,1024] add).  The host un-permutes the
    group-local u-major column order during the gather.
  - Setup matmuls accumulate k-OUTER into two wide PSUM tiles (only
    the bank's first matmul uses start=True) so they run as each input
    DMA chunk lands.
  - Tapered group sizes [4,12,16*6,12,4] shorten pipeline fill/drain.
  - 6 input DMAs (4 chunk-interleaved blobs + wout + biases) split
    across the Sync/GpSimd descriptor-gen queues; each row block's 4
    vq outputs merge into ONE strided-AP DMA (descriptor-gen on the
    Sync sequencer costs ~650ns serial per DMA).
  - ACT tanh table preloaded with a dummy tanh at t~0; all z-path and
    matmul operands bf16 (host pre-casts), bf16 output (host upcasts).
Known-bad variants (measured): GpSimd tensor ops alongside DVE
perf-mode work (shared-port lockstep stall); stride-0-innermost bf16
adds (1.5us slow path); K=1 bias matmuls on PE (320ns each); K=1 HAM
pre-warm matmuls (427ns each, block the in-order PE stream).
"""

import numpy as np

B, T, U = 4, 256, 64
D, J, V = 512, 512, 512
NCORES = 8
TH = T // 2          # t's per core = 128
KC = 4               # 512/128 contraction chunks
VQ = 4               # v-quarters (output partition chunks)
MAXTG = 16

# ---- tuning knobs ----
Z_FP32 = False       # zpre dtype fp32 (True) or bf16 (False; enables 2x adds)
GP_JC = set()        # zpre adds for these jc run on GpSimd instead of DVE
GROUP_T = [4, 12] + [16] * 6 + [12, 4]
assert sum(GROUP_T) == TH
N_DIRECT = 0         # first N groups use direct decT-broadcast adds

_compiled = None

# blob1: per contraction chunk k: henct_k | hdect_k | wenc_k | wdec_k (bf16)
B1_K = TH + U + 2 * J            # 1216 cols per chunk
B1_COLS = KC * B1_K
# blob2: wout (bf16), stationary chunks
B2_COLS = KC * V
# blob3: benc | boutp  (fp32)
B3_COLS = 2 * KC


def _build():
    import concourse.bass as bass
    import concourse.tile as tile
    from concourse import mybir

    fp32 = mybir.dt.float32
    bf16 = mybir.dt.bfloat16
    AF = mybir.ActivationFunctionType
    zdt = fp32 if Z_FP32 else bf16

    nc = bass.Bass()

    blob1 = nc.declare_dram_parameter("blob1", [128, B1_COLS], bf16, isOutput=False)
    blob2 = nc.declare_dram_parameter("blob2", [128, B2_COLS], bf16, isOutput=False)
    blob3 = nc.declare_dram_parameter("blob3", [128, B3_COLS], fp32, isOutput=False)
    out = nc.declare_dram_parameter("out", [V, TH * U], bf16, isOutput=True)

    with tile.TileContext(nc) as tc:
        with (
            tc.tile_pool(name="const", bufs=1) as const,
            tc.tile_pool(name="zpre", bufs=5) as zpre_pool,
            tc.tile_pool(name="zt", bufs=5) as zt_pool,
            tc.tile_pool(name="outs", bufs=8) as outs_pool,
            tc.tile_pool(name="ps_out", bufs=8, space="PSUM") as ps_out,
        ):
            # ---- ACT table warmup (pool tiles are zero-inited) ----
            warm = const.tile([1, 2], fp32, tag="warm")
            nc.scalar.activation(warm[0:1, 1:2], warm[0:1, 0:1], AF.Tanh)

            # ---- load everything to SBUF; blob1 split into one DMA (and
            # one tile) per contraction chunk so setup matmuls start as
            # soon as the first chunk lands ----
            # alternate input DMAs between the Sync and (otherwise idle)
            # GpSimd descriptor-gen queues so the serial ~650ns per-DMA
            # DIRECT2D cost overlaps (the transfer itself uses the AXI
            # ports, so no shared-engine-port hazard)
            b1k = []
            for k in range(KC):
                t = const.tile([128, B1_K], bf16, tag=f"b1k{k}")
                eng = nc.sync if k % 2 == 0 else nc.gpsimd
                eng.dma_start(t[:], blob1[:, k * B1_K:(k + 1) * B1_K])
                b1k.append(t)
            b2 = const.tile([128, B2_COLS], bf16, tag="b2")
            nc.gpsimd.dma_start(b2[:], blob2[:])
            b3 = const.tile([128, B3_COLS], fp32, tag="b3")
            nc.gpsimd.dma_start(b3[:], blob3[:])

            def henct_k(k):
                return b1k[k][:, 0:TH]

            def hdect_k(k):
                return b1k[k][:, TH:TH + U]

            def wenc_kj(k, jc):
                c = TH + U + jc * 128
                return b1k[k][:, c:c + 128]

            def wdec_kj(k, jc):
                c = TH + U + J + jc * 128
                return b1k[k][:, c:c + 128]

            def wout_jv(jc, vq):
                c = (jc * VQ + vq) * 128
                return b2[:, c:c + 128]

            benc_s = b3[:, 0:KC]
            boutp_s = b3[:, KC:2 * KC]

            # ---- encT / decT setup matmuls, k-OUTER: the 4 jc regions of
            # enc (and dec) accumulate in two wide PSUM tiles so chunk-k
            # matmuls run as each input DMA chunk lands instead of every
            # jc waiting for all four chunks.  Only the first matmul into
            # each bank uses start=True (clears has_written bank-wide);
            # every other region's first write lands on a cleared bit and
            # overwrites, later k's accumulate.  Evacs on ACT. ----
            # the setup accumulators are the first two slots of the main
            # ps_out pool rotation: after their ACT evacs (~19us) the banks
            # recycle into the main loop, which then pipelines on all 8
            # PSUM banks instead of 6.  dec before enc per chunk: decT
            # gates the DVE dec_repU chain.
            pd_all = ps_out.tile([128, 512], fp32, tag="po")
            pe_all = ps_out.tile([128, KC * TH], fp32, tag="po")
            for k in range(KC):
                for jc in range(KC):
                    nc.tensor.matmul(
                        pd_all[:, jc * U:(jc + 1) * U],
                        wdec_kj(k, jc),
                        hdect_k(k),
                        start=(k == 0 and jc == 0),
                        stop=(k == KC - 1 and jc == KC - 1),
                        skip_group_check=True,
                    )
                for jc in range(KC):
                    nc.tensor.matmul(
                        pe_all[:, jc * TH:(jc + 1) * TH],
                        wenc_kj(k, jc),
                        henct_k(k),
                        start=(k == 0 and jc == 0),
                        stop=(k == KC - 1 and jc == KC - 1),
                        skip_group_check=True,
                    )
            encT_s = []
            decT_s = []
            for jc in range(KC):
                et = const.tile([128, TH], zdt, tag=f"encT{jc}")
                nc.scalar.activation(
                    et[:], pe_all[:, jc * TH:(jc + 1) * TH],
                    AF.Identity, bias=benc_s[:, jc:jc + 1],
                )
                encT_s.append(et)
                dt_ = const.tile([128, U], zdt, tag=f"decT{jc}")
                nc.scalar.activation(
                    dt_[:], pd_all[:, jc * U:(jc + 1) * U], AF.Copy
                )
                decT_s.append(dt_)

            # ---- dec_repU[jc][j, u, t] = decT[jc][j, u]  (one-time DVE
            # 1x copies; they hide behind the enc/dec setup-matmul phase.
            # NOT GpSimd: a long GpSimd op holds the shared DVE/GpSimd SBUF
            # port and lockstep-stalls every concurrent DVE perf-mode op.) ----
            dec_repU = []
            for jc in range(KC):
                dr = const.tile([128, U * MAXTG], zdt, tag=f"drepU{jc}")
                dr3 = dr[:].rearrange("p (u t) -> p u t", u=U)
                nc.vector.tensor_copy(
                    dr3,
                    decT_s[jc][:]
                    .rearrange("p (u x) -> p u x", x=1)
                    .to_broadcast([128, U, MAXTG]),
                )
                dec_repU.append(dr)

            # ---- main loop ----
            evac_ctr = 0
            col0 = 0
            for g, tg in enumerate(GROUP_T):
                gw = tg * U          # z columns this group covers
                zp = zpre_pool.tile([128, KC * MAXTG * U], zdt, tag="zp")
                for jc in range(KC):
                    zps = zp[:, jc * gw:(jc + 1) * gw]
                    zp3 = zps.rearrange("p (u t) -> p u t", u=U)
                    if g < N_DIRECT:
                        d3 = (
                            decT_s[jc][:]
                            .rearrange("p (u x) -> p u x", x=1)
                            .to_broadcast([128, U, tg])
                        )
                    else:
                        d3 = (
                            dec_repU[jc][:]
                            .rearrange("p (u t) -> p u t", u=U)[:, :, 0:tg]
                        )
                    e3 = (
                        encT_s[jc][:, col0 // U:col0 // U + tg]
                        .rearrange("p (x t) -> p x t", x=1)
                        .to_broadcast([128, U, tg])
                    )
                    eng = nc.gpsimd if jc in GP_JC else nc.vector
                    eng.tensor_add(zp3, d3, e3)
                zt = zt_pool.tile([128, KC * MAXTG * U], bf16, tag="zt")
                nc.scalar.activation(zt[:, :KC * gw], zp[:, :KC * gw], AF.Tanh)

                # split the group's columns into <=512-wide row blocks;
                # each block's 4 vq outputs share one wide ob tile and ONE
                # DMA (the serial ~600ns/DMA DIRECT2D descriptor-gen on the
                # Sync sequencer was stretching both steady state and drain)
                widths = [512] * (gw // 512)
                if gw % 512:
                    widths.append(gw % 512)
                rc = 0
                for w in widths:
                    ob = outs_pool.tile([128, VQ * 512], bf16, tag="ob")
                    for vq in range(VQ):
                        po = ps_out.tile([128, 512], fp32, tag="po")
                        for jc in range(KC):
                            nc.tensor.matmul(
                                po[:, 0:w],
                                wout_jv(jc, vq),
                                zt[:, jc * gw + rc:jc * gw + rc + w],
                                start=(jc == 0),
                                stop=(jc == KC - 1),
                            )
                        obs = ob[:, vq * 512:vq * 512 + w]
                        if (evac_ctr * 3) % 8 < 3:
                            nc.scalar.activation(
                                obs, po[:, 0:w], AF.Identity,
                                bias=boutp_s[:, vq:vq + 1],
                            )
                        else:
                            nc.vector.tensor_scalar_add(
                                obs, po[:, 0:w], boutp_s[:, vq:vq + 1]
                            )
                        evac_ctr += 1
                    c0 = col0 + rc
                    nc.sync.dma_start(
                        out[:, c0:c0 + w].rearrange("(v p) c -> p v c", v=VQ),
                        ob[:].rearrange("p (v c) -> p v c", v=VQ)[:, :, 0:w],
                    )
                    rc += w
                col0 += gw

    _split_multi_waits(nc)
    return nc


_COMPUTE_OPS = {
    "Matmult", "Ldweights", "TensorTensor", "TensorCopy", "TensorScalarPtr",
    "Activation", "TensorReduce", "Memset", "ScalarTensorTensor",
    "TensorScalar", "DMACopy", "Drain", "EventSemaphore",
}


def _split_multi_waits(nc):
    """walrus codegen in this container allows a single sync-wait command
    per TPB compute instruction; Tile emits several.  Hoist all but one
    wait onto standalone EventSemaphore instructions placed just before
    the offending instruction (same engine, so semantics are identical).
    """
    from concourse import mybir

    ctr = [0]
    for fn in nc.m.functions:
        for blk in fn.blocks:
            insts = blk.instructions
            out = []
            for inst in insts:
                si = getattr(inst, "sync_info", None)
                ow = list(si.on_wait) if si and si.on_wait else []
                if (
                    len(ow) > 1
                    and getattr(inst, "opcode", None) in _COMPUTE_OPS
                ):
                    for w in ow[:-1]:
                        ctr[0] += 1
                        ev = mybir.InstEventSemaphore(
                            name=f"WS-{ctr[0]}-{inst.name}",
                            ins=[],
                            outs=[],
                            sync_info=mybir.SyncInfo(
                                on_wait=[w], on_update=[]
                            ),
                        )
                        ev.engine = inst.engine
                        out.append(ev)
                    inst.sync_info = mybir.SyncInfo(
                        on_wait=[ow[-1]], on_update=list(si.on_update or [])
                    )
                out.append(inst)
            blk.instructions = out


def _get_compiled():
    global _compiled
    if _compiled is None:
        _compiled = _build()
    return _compiled


def _chunk_rows(mat, ncols):
    """[D, N] (contraction-major) -> [128, KC*N] f32, chunk-interleaved:
    out[p, k*N + n] = mat[k*128 + p, n]"""
    m = np.asarray(mat, dtype=np.float32).reshape(KC, 128, ncols)
    return m.transpose(1, 0, 2).reshape(128, KC * ncols)


def kernel(h_enc, h_dec, W_enc, b_enc, W_dec, W_out, b_out, **_):
    import ml_dtypes

    nc = _get_compiled()
    from concourse.bass_utils import run_bass_kernel_spmd

    bf16 = ml_dtypes.bfloat16
    h_enc = np.asarray(h_enc, dtype=np.float32)
    h_dec = np.asarray(h_dec, dtype=np.float32)

    # per-chunk [128, N] views, chunk-major lists
    wenc_k = _chunk_rows(W_enc, J).reshape(128, KC, J)
    wdec_k = _chunk_rows(W_dec, J).reshape(128, KC, J)
    blob2 = np.ascontiguousarray(
        np.asarray(W_out, dtype=np.float32)
        .reshape(KC, 128, VQ, 128)
        .transpose(1, 0, 2, 3)
        .reshape(128, KC * V)
        .astype(bf16)
    )
    blob3 = np.ascontiguousarray(
        np.concatenate(
            [
                np.asarray(b_enc, dtype=np.float32).reshape(KC, 128).T,
                np.asarray(b_out, dtype=np.float32).reshape(VQ, 128).T,
            ],
            axis=1,
        )
    )

    hdect_b = {
        b: _chunk_rows(h_dec[b, 0, :, :].T, U).reshape(128, KC, U)
        for b in range(B)
    }
    in_maps = []
    for c in range(NCORES):
        b, th = c // 2, c % 2
        henct_k = _chunk_rows(
            h_enc[b, th * TH:(th + 1) * TH, 0, :].T, TH
        ).reshape(128, KC, TH)
        # blob1: per chunk k: henct_k | hdect_k | wenc_k | wdec_k
        parts = []
        for k in range(KC):
            parts.extend(
                [henct_k[:, k], hdect_b[b][:, k], wenc_k[:, k], wdec_k[:, k]]
            )
        blob1 = np.ascontiguousarray(np.concatenate(parts, axis=1).astype(bf16))
        in_maps.append({"blob1": blob1, "blob2": blob2, "blob3": blob3})

    global _last_in_maps
    _last_in_maps = in_maps
    res = run_bass_kernel_spmd(nc, in_maps, list(range(NCORES)))

    out_full = np.empty((B, T, U, V), dtype=np.float32)
    for c in range(NCORES):
        b, th = c // 2, c % 2
        outT = np.asarray(res.results[c]["out"]).astype(np.float32)  # [V, 8192]
        t0 = th * TH
        col0 = 0
        for tg in GROUP_T:
            gw = tg * U
            blk = outT[:, col0:col0 + gw].reshape(V, U, tg)
            # [v, u, t] -> [t, u, v]
            out_full[b, t0:t0 + tg] = blk.transpose(2, 1, 0)
            t0 += tg
            col0 += gw
    return out_full


# revision 37
# speedup vs baseline: 1.0107x; 1.0107x over previous
"""Trainium2 Bass kernel for the RNN-T JointNetwork problem.

  enc = h_enc @ W_enc + b_enc            (B,T,1,J)
  dec = h_dec @ W_dec                    (B,1,U,J)
  z   = tanh(enc + dec)                  (B,T,U,J)
  out = z @ W_out + b_out                (B,T,U,V)

Shapes: B=4, T=256, U=64, D=J=V=512, fp32 in/out.

Sharding: 8 cores, data parallel over (B x T/2): core c handles batch
b = c//2 and t-half th = c%2 (128 t values). Params replicated.

Final design, ~83us/core measured (vs 122us baseline); rel err 4.1e-3:
  - TRANSPOSED OUTPUT (outT[v, row]): W_out chunks stationary, zT
    moving, b_out per-partition -> evacuation via DVE tensor_scalar_add
    / ACT Identity-with-bias (5:3 split), PE does only the 256+32 main
    matmuls (TensorMatrix 99-101% busy in steady state).
  - U-MAJOR z rows within each t-group: row = u*tg + t_local.  With
    dec_repU[j, u, t] = decT[j, u] pre-replicated (one-time DVE copies
    that hide behind the setup-matmul phase), both zpre-add operands
    are innermost-step-1 bf16 APs, which unlocks the DVE 2x_1P mode
    (600ns vs 1133ns per [128,1024] add).  The host un-permutes the
    group-local u-major column order during the gather.
  - Setup matmuls accumulate k-OUTER into two wide PSUM tiles (only
    the bank's first matmul uses start=True) so they run as each input
    DMA chunk lands.
  - Tapered group sizes [4,12,16*6,12,4] shorten pipeline fill/drain.
  - 6 input DMAs (4 chunk-interleaved blobs + wout + biases) split
    across the Sync/GpSimd descriptor-gen queues; each row block's 4
    vq outputs merge into ONE strided-AP DMA (descriptor-gen on the
    Sync sequencer costs ~650ns serial per DMA).
  - ACT tanh table preloaded with a dummy tanh at t~0; all z-path and
    matmul operands bf16 (host pre-casts), bf16 output (host upcasts).
Known-bad variants (measured): GpSimd tensor ops alongside DVE
perf-mode work (shared-port lockstep stall); stride-0-innermost bf16
adds (1.5us slow path); K=1 bias matmuls on PE (320ns each); K=1 HAM
pre-warm matmuls (427ns each, block the in-order PE stream).
"""

import numpy as np

B, T, U = 4, 256, 64
D, J, V = 512, 512, 512
NCORES = 8
TH = T // 2          # t's per core = 128
KC = 4               # 512/128 contraction chunks
VQ = 4               # v-quarters (output partition chunks)
MAXTG = 16

# ---- tuning knobs ----
Z_FP32 = False       # zpre dtype fp32 (True) or bf16 (False; enables 2x adds)
GP_JC = set()        # zpre adds for these jc run on GpSimd instead of DVE
GROUP_T = [4, 12] + [16] * 6 + [12, 4]
assert sum(GROUP_T) == TH
N_DIRECT = 0         # first N groups use direct decT-broadcast adds

_compiled = None

# blob1: per contraction chunk k: henct_k | hdect_k | wenc_k | wdec_k (bf16)
B1_K = TH + U + 2 * J            # 1216 cols per chunk
B1_COLS = KC * B1_K
# blob2: wout (bf16), stationary chunks
B2_COLS = KC * V
# blob3: benc | boutp  (fp32)
B3_COLS = 2 * KC


def _build():
    import concourse.bass as bass
    import concourse.tile as tile
    from concourse import mybir

    fp32 = mybir.dt.float32
    bf16 = mybir.dt.bfloat16
    AF = mybir.ActivationFunctionType
    zdt = fp32 if Z_FP32 else bf16

    nc = bass.Bass()

    blob1 = nc.declare_dram_parameter("blob1", [128, B1_COLS], bf16, isOutput=False)
    blob2 = nc.declare_dram_parameter("blob2", [128, B2_COLS], bf16, isOutput=False)
    blob3 = nc.declare_dram_parameter("blob3", [128, B3_COLS], fp32, isOutput=False)
    out = nc.declare_dram_parameter("out", [V, TH * U], bf16, isOutput=True)

    with tile.TileContext(nc) as tc:
        with (
            tc.tile_pool(name="const", bufs=1) as const,
            tc.tile_pool(name="zpre", bufs=5) as zpre_pool,
            tc.tile_pool(name="zt", bufs=5) as zt_pool,
            tc.tile_pool(name="outs", bufs=8) as outs_pool,
            tc.tile_pool(name="ps_setup", bufs=1, space="PSUM") as ps_setup,
            tc.tile_pool(name="ps_out", bufs=6, space="PSUM") as ps_out,
        ):
            # ---- ACT table warmup (pool tiles are zero-inited) ----
            warm = const.tile([1, 2], fp32, tag="warm")
            nc.scalar.activation(warm[0:1, 1:2], warm[0:1, 0:1], AF.Tanh)

            # ---- load everything to SBUF; blob1 split into one DMA (and
            # one tile) per contraction chunk so setup matmuls start as
            # soon as the first chunk lands ----
            # alternate input DMAs between the Sync and (otherwise idle)
            # GpSimd descriptor-gen queues so the serial ~650ns per-DMA
            # DIRECT2D cost overlaps (the transfer itself uses the AXI
            # ports, so no shared-engine-port hazard)
            b1k = []
            for k in range(KC):
                t = const.tile([128, B1_K], bf16, tag=f"b1k{k}")
                eng = nc.sync if k % 2 == 0 else nc.gpsimd
                eng.dma_start(t[:], blob1[:, k * B1_K:(k + 1) * B1_K])
                b1k.append(t)
            b2 = const.tile([128, B2_COLS], bf16, tag="b2")
            nc.gpsimd.dma_start(b2[:], blob2[:])
            b3 = const.tile([128, B3_COLS], fp32, tag="b3")
            nc.gpsimd.dma_start(b3[:], blob3[:])

            def henct_k(k):
                return b1k[k][:, 0:TH]

            def hdect_k(k):
                return b1k[k][:, TH:TH + U]

            def wenc_kj(k, jc):
                c = TH + U + jc * 128
                return b1k[k][:, c:c + 128]

            def wdec_kj(k, jc):
                c = TH + U + J + jc * 128
                return b1k[k][:, c:c + 128]

            def wout_jv(jc, vq):
                c = (jc * VQ + vq) * 128
                return b2[:, c:c + 128]

            benc_s = b3[:, 0:KC]
            boutp_s = b3[:, KC:2 * KC]

            # ---- encT / decT setup matmuls, k-OUTER: the 4 jc regions of
            # enc (and dec) accumulate in two wide PSUM tiles so chunk-k
            # matmuls run as each input DMA chunk lands instead of every
            # jc waiting for all four chunks.  Only the first matmul into
            # each bank uses start=True (clears has_written bank-wide);
            # every other region's first write lands on a cleared bit and
            # overwrites, later k's accumulate.  Evacs on ACT. ----
            pe_all = ps_setup.tile([128, KC * TH], fp32, tag="pse")
            pd_all = ps_setup.tile([128, KC * U], fp32, tag="psd")
            for k in range(KC):
                for jc in range(KC):
                    nc.tensor.matmul(
                        pe_all[:, jc * TH:(jc + 1) * TH],
                        wenc_kj(k, jc),
                        henct_k(k),
                        start=(k == 0 and jc == 0),
                        stop=(k == KC - 1 and jc == KC - 1),
                        skip_group_check=True,
                    )
                for jc in range(KC):
                    nc.tensor.matmul(
                        pd_all[:, jc * U:(jc + 1) * U],
                        wdec_kj(k, jc),
                        hdect_k(k),
                        start=(k == 0 and jc == 0),
                        stop=(k == KC - 1 and jc == KC - 1),
                        skip_group_check=True,
                    )
            encT_s = []
            decT_s = []
            for jc in range(KC):
                et = const.tile([128, TH], zdt, tag=f"encT{jc}")
                nc.scalar.activation(
                    et[:], pe_all[:, jc * TH:(jc + 1) * TH],
                    AF.Identity, bias=benc_s[:, jc:jc + 1],
                )
                encT_s.append(et)
                dt_ = const.tile([128, U], zdt, tag=f"decT{jc}")
                nc.scalar.activation(
                    dt_[:], pd_all[:, jc * U:(jc + 1) * U], AF.Copy
                )
                decT_s.append(dt_)

            # ---- dec_repU[jc][j, u, t] = decT[jc][j, u]  (one-time DVE
            # 1x copies; they hide behind the enc/dec setup-matmul phase.
            # NOT GpSimd: a long GpSimd op holds the shared DVE/GpSimd SBUF
            # port and lockstep-stalls every concurrent DVE perf-mode op.) ----
            dec_repU = []
            for jc in range(KC):
                dr = const.tile([128, U * MAXTG], zdt, tag=f"drepU{jc}")
                dr3 = dr[:].rearrange("p (u t) -> p u t", u=U)
                nc.vector.tensor_copy(
                    dr3,
                    decT_s[jc][:]
                    .rearrange("p (u x) -> p u x", x=1)
                    .to_broadcast([128, U, MAXTG]),
                )
                dec_repU.append(dr)

            # ---- main loop ----
            evac_ctr = 0
            col0 = 0
            for g, tg in enumerate(GROUP_T):
                gw = tg * U          # z columns this group covers
                zp = zpre_pool.tile([128, KC * MAXTG * U], zdt, tag="zp")
                for jc in range(KC):
                    zps = zp[:, jc * gw:(jc + 1) * gw]
                    zp3 = zps.rearrange("p (u t) -> p u t", u=U)
                    if g < N_DIRECT:
                        d3 = (
                            decT_s[jc][:]
                            .rearrange("p (u x) -> p u x", x=1)
                            .to_broadcast([128, U, tg])
                        )
                    else:
                        d3 = (
                            dec_repU[jc][:]
                            .rearrange("p (u t) -> p u t", u=U)[:, :, 0:tg]
                        )
                    e3 = (
                        encT_s[jc][:, col0 // U:col0 // U + tg]
                        .rearrange("p (x t) -> p x t", x=1)
                        .to_broadcast([128, U, tg])
                    )
                    eng = nc.gpsimd if jc in GP_JC else nc.vector
                    eng.tensor_add(zp3, d3, e3)
                zt = zt_pool.tile([128, KC * MAXTG * U], bf16, tag="zt")
                nc.scalar.activation(zt[:, :KC * gw], zp[:, :KC * gw], AF.Tanh)

                # split the group's columns into <=512-wide row blocks;
                # each block's 4 vq outputs share one wide ob tile and ONE
                # DMA (the serial ~600ns/DMA DIRECT2D descriptor-gen on the
                # Sync sequencer was stretching both steady state and drain)
                widths = [512] * (gw // 512)
                if gw % 512:
                    widths.append(gw % 512)
                rc = 0
                for w in widths:
                    ob = outs_pool.tile([128, VQ * 512], bf16, tag="ob")
                    for vq in range(VQ):
                        po = ps_out.tile([128, 512], fp32, tag="po")
                        for jc in range(KC):
                            nc.tensor.matmul(
                                po[:, 0:w],
                                wout_jv(jc, vq),
                                zt[:, jc * gw + rc:jc * gw + rc + w],
                                start=(jc == 0),
                                stop=(jc == KC - 1),
                            )
                        obs = ob[:, vq * 512:vq * 512 + w]
                        if (evac_ctr * 3) % 8 < 3:
                            nc.scalar.activation(
                                obs, po[:, 0:w], AF.Identity,
                                bias=boutp_s[:, vq:vq + 1],
                            )
                        else:
                            nc.vector.tensor_scalar_add(
                                obs, po[:, 0:w], boutp_s[:, vq:vq + 1]
                            )
                        evac_ctr += 1
                    c0 = col0 + rc
                    nc.sync.dma_start(
                        out[:, c0:c0 + w].rearrange("(v p) c -> p v c", v=VQ),
                        ob[:].rearrange("p (v c) -> p v c", v=VQ)[:, :, 0:w],
                    )
                    rc += w
                col0 += gw

    _split_multi_waits(nc)
    return nc


_COMPUTE_OPS = {
    "Matmult", "Ldweights", "TensorTensor", "TensorCopy", "TensorScalarPtr",
    "Activation", "TensorReduce", "Memset", "ScalarTensorTensor",
    "TensorScalar", "DMACopy", "Drain", "EventSemaphore",
}


def _split_multi_waits(nc):
    """walrus codegen in this container allows a single sync-wait command
    per TPB compute instruction; Tile emits several.  Hoist all but one
    wait onto standalone EventSemaphore instructions placed just before
    the offending instruction (same engine, so semantics are identical).
    """
    from concourse import mybir

    ctr = [0]
    for fn in nc.m.functions:
        for blk in fn.blocks:
            insts = blk.instructions
            out = []
            for inst in insts:
                si = getattr(inst, "sync_info", None)
                ow = list(si.on_wait) if si and si.on_wait else []
                if (
                    len(ow) > 1
                    and getattr(inst, "opcode", None) in _COMPUTE_OPS
                ):
                    for w in ow[:-1]:
                        ctr[0] += 1
                        ev = mybir.InstEventSemaphore(
                            name=f"WS-{ctr[0]}-{inst.name}",
                            ins=[],
                            outs=[],
                            sync_info=mybir.SyncInfo(
                                on_wait=[w], on_update=[]
                            ),
                        )
                        ev.engine = inst.engine
                        out.append(ev)
                    inst.sync_info = mybir.SyncInfo(
                        on_wait=[ow[-1]], on_update=list(si.on_update or [])
                    )
                out.append(inst)
            blk.instructions = out


def _get_compiled():
    global _compiled
    if _compiled is None:
        _compiled = _build()
    return _compiled


def _chunk_rows(mat, ncols):
    """[D, N] (contraction-major) -> [128, KC*N] f32, chunk-interleaved:
    out[p, k*N + n] = mat[k*128 + p, n]"""
    m = np.asarray(mat, dtype=np.float32).reshape(KC, 128, ncols)
    return m.transpose(1, 0, 2).reshape(128, KC * ncols)


def kernel(h_enc, h_dec, W_enc, b_enc, W_dec, W_out, b_out, **_):
    import ml_dtypes

    nc = _get_compiled()
    from concourse.bass_utils import run_bass_kernel_spmd

    bf16 = ml_dtypes.bfloat16
    h_enc = np.asarray(h_enc, dtype=np.float32)
    h_dec = np.asarray(h_dec, dtype=np.float32)

    # per-chunk [128, N] views, chunk-major lists
    wenc_k = _chunk_rows(W_enc, J).reshape(128, KC, J)
    wdec_k = _chunk_rows(W_dec, J).reshape(128, KC, J)
    blob2 = np.ascontiguousarray(
        np.asarray(W_out, dtype=np.float32)
        .reshape(KC, 128, VQ, 128)
        .transpose(1, 0, 2, 3)
        .reshape(128, KC * V)
        .astype(bf16)
    )
    blob3 = np.ascontiguousarray(
        np.concatenate(
            [
                np.asarray(b_enc, dtype=np.float32).reshape(KC, 128).T,
                np.asarray(b_out, dtype=np.float32).reshape(VQ, 128).T,
            ],
            axis=1,
        )
    )

    hdect_b = {
        b: _chunk_rows(h_dec[b, 0, :, :].T, U).reshape(128, KC, U)
        for b in range(B)
    }
    in_maps = []
    for c in range(NCORES):
        b, th = c // 2, c % 2
        henct_k = _chunk_rows(
            h_enc[b, th * TH:(th + 1) * TH, 0, :].T, TH
        ).reshape(128, KC, TH)
        # blob1: per chunk k: henct_k | hdect_k | wenc_k | wdec_k
        parts = []
        for k in range(KC):
            parts.extend(
                [henct_k[:, k], hdect_b[b][:, k], wenc_k[:, k], wdec_k[:, k]]
            )
        blob1 = np.ascontiguousarray(np.concatenate(parts, axis=1).astype(bf16))
        in_maps.append({"blob1": blob1, "blob2": blob2, "blob3": blob3})

    global _last_in_maps
    _last_in_maps = in_maps
    res = run_bass_kernel_spmd(nc, in_maps, list(range(NCORES)))

    out_full = np.empty((B, T, U, V), dtype=np.float32)
    for c in range(NCORES):
        b, th = c // 2, c % 2
        outT = np.asarray(res.results[c]["out"]).astype(np.float32)  # [V, 8192]
        t0 = th * TH
        col0 = 0
        for tg in GROUP_T:
            gw = tg * U
            blk = outT[:, col0:col0 + gw].reshape(V, U, tg)
            # [v, u, t] -> [t, u, v]
            out_full[b, t0:t0 + tg] = blk.transpose(2, 1, 0)
            t0 += tg
            col0 += gw
    return out_full
